# revision 9
# baseline (speedup 1.0000x reference)
"""MoE FFN (8 routed experts top-2 + 1 shared) on 8 TRN2 NeuronCores.

Expert-parallel token dispatch (see kernel3 docstring). This version
pipelines the routed expert in two slot halves so the first AllToAll return
overlaps the second half's compute, streams weight tiles through small
reused SBUF windows so the shared-expert loads overlap the routed phase,
and takes the gathered tokens in both fp32 (router precision) and bf16
(matmul operand) form from the host sharding layer.
"""

import ml_dtypes
import numpy as np

import concourse.bacc as bacc
import concourse.bass as bass
import concourse.mybir as mybir
import concourse.tile as tile
from concourse.bass_utils import run_bass_kernel_spmd

P = 128
C = 1024            # d_model
H = 2048            # d_expert
T = 4096            # tokens
E = 8               # routed experts == cores
TS = T // E         # 512 tokens per core
CC = C // P         # 8
HC = H // P         # 16
LC = 144            # per (expert, owner) capacity (host balances ownership)
S = E * LC          # 1280 slots per expert core
SCH = S // P        # 10 slot chunks
JH = 64             # per-owner slots in a2a half 0 (half 1: LC-JH)
SH = E * JH         # 512 rows in a2a half 0
NHB = 4             # 512-wide h blocks
BIG = 1.0e9

F32 = mybir.dt.float32
BF16 = mybir.dt.bfloat16

NCORES = 8
AX = mybir.AxisListType
OP = mybir.AluOpType
ACTF = mybir.ActivationFunctionType


def _ap3(t, dims):
    a = t[:]
    return bass.AP(a.tensor, a.offset, [list(a.ap[0])] + dims)


def _build_program():
    nc = bacc.Bacc("TRN2", target_bir_lowering=False, debug=False,
                   num_devices=NCORES)

    xgT = nc.dram_tensor("xgT", [C, S], F32, kind="ExternalInput")
    xgTb = nc.dram_tensor("xgTb", [C, S], BF16, kind="ExternalInput")
    rwu = nc.dram_tensor("rwu", [C, H], BF16, kind="ExternalInput")
    rwg = nc.dram_tensor("rwg", [C, H], BF16, kind="ExternalInput")
    rwd = nc.dram_tensor("rwd", [H, C], BF16, kind="ExternalInput")
    swu = nc.dram_tensor("swu", [C, H], BF16, kind="ExternalInput")
    swg = nc.dram_tensor("swg", [C, H], BF16, kind="ExternalInput")
    swd = nc.dram_tensor("swd", [H, C], BF16, kind="ExternalInput")
    xsTb = nc.dram_tensor("xsTb", [C, TS], BF16, kind="ExternalInput")
    rtw = nc.dram_tensor("rtw", [C, E], F32, kind="ExternalInput")
    ohx = nc.dram_tensor("ohx", [P, E], F32, kind="ExternalInput")
    qm = nc.dram_tensor("qm", [S, TS], BF16, kind="ExternalInput")

    out = nc.dram_tensor("out", [TS, C], F32, kind="ExternalOutput")

    a2a_in = nc.dram_tensor("a2a_in", [S, C], BF16)
    a2a_out = nc.dram_tensor("a2a_out", [S, C], BF16)

    with tile.TileContext(nc) as tc:
        with (
            tc.tile_pool(name="pers", bufs=1) as pers,
            tc.tile_pool(name="psm", bufs=1, space="PSUM") as psm,
        ):
            rt = [pers.tile([P, E], F32, tag=f"rt{cc}", name=f"rt{cc}")
                  for cc in range(CC)]
            oh = pers.tile([P, E], F32, tag="oh", name="oh")
            wslot = pers.tile([P, SCH], F32, tag="wslot", name="wslot")


            def ps_up():
                return psm.tile([P, 512], F32, tag="up", bufs=2, name="up")

            def ps_gt():
                return psm.tile([P, 512], F32, tag="gt", bufs=2, name="gt")

            def ps_y():
                return psm.tile([P, 512], F32, tag="y", bufs=2, name="y")

            with tc.tile_pool(name="pC", bufs=1) as pc_:
                actr = [pc_.tile([P, S], BF16, tag=f"ar{hc}", name=f"ar{hc}")
                        for hc in range(HC)]
                rdb = [pc_.tile([P, C], BF16, tag=f"rwd{hc}", name=f"rwd{hc}")
                       for hc in range(HC)]
                acts = [pc_.tile([P, TS], BF16, tag=f"as{hc}", name=f"as{hc}")
                        for hc in range(HC)]
                with tc.tile_pool(name="pB", bufs=1) as pb:
                    xgtb = [pb.tile([P, S], BF16, tag=f"xgtb{cc}",
                                    name=f"xgtb{cc}") for cc in range(CC)]
                    # streamed rwu/rwg window: 2 hb-blocks resident
                    rwin = [[pb.tile([P, 512], BF16, tag=f"rw{cc}_{par}",
                                     name=f"rw{cc}_{par}")
                             for par in range(2)] for cc in range(CC)]
                    gwin = [[pb.tile([P, 512], BF16, tag=f"gw{cc}_{par}",
                                     name=f"gw{cc}_{par}")
                             for par in range(2)] for cc in range(CC)]
                    for cc in range(CC):
                        eng = nc.sync if cc % 2 == 0 else nc.scalar
                        eng.dma_start(xgtb[cc][:, 0:SH],
                                      xgTb[cc * P:(cc + 1) * P, 0:SH])
                    for cc in range(CC):
                        nc.gpsimd.dma_start(xgtb[cc][:, SH:S],
                                            xgTb[cc * P:(cc + 1) * P, SH:S])

                    def load_rw(hb, split=False):
                        for half in ([0, 1] if split else [None]):
                            for cc in range(CC):
                                cl = slice(hb * 512, (hb + 1) * 512) \
                                    if half is None else \
                                    slice(hb * 512 + half * 256,
                                          hb * 512 + (half + 1) * 256)
                                tl = slice(0, 512) if half is None else \
                                    slice(half * 256, (half + 1) * 256)
                                nc.sync.dma_start(
                                    rwin[cc][hb % 2][:, tl],
                                    rwu[cc * P:(cc + 1) * P, cl])
                                nc.sync.dma_start(
                                    gwin[cc][hb % 2][:, tl],
                                    rwg[cc * P:(cc + 1) * P, cl])

                    for cc in range(CC):
                        nc.sync.dma_start(rwin[cc][0][:],
                                          rwu[cc * P:(cc + 1) * P, 0:512])
                        nc.scalar.dma_start(gwin[cc][0][:],
                                            rwg[cc * P:(cc + 1) * P, 0:512])
                    load_rw(1)

                    def upgate_half(h, lo, hi):
                        for hc in range(HC):
                            hb, hj = hc // 4, (hc % 4) * P
                            if hc % 4 == 0 and hc >= 4 and hb + 1 >= 2 \
                                    and hb + 1 <= 3:
                                load_rw(hb + 1)
                            mid = (lo + hi) // 2
                            for sub, (a, b) in enumerate([(lo, mid),
                                                          (mid, hi)]):
                                n = b - a
                                up_ps, gt_ps = ps_up(), ps_gt()
                                for cc in range(CC):
                                    nc.tensor.matmul(
                                        up_ps[:, 0:n],
                                        rwin[cc][hb % 2][:, hj:hj + P],
                                        xgtb[cc][:, a:b], start=(cc == 0),
                                        stop=(cc == CC - 1))
                                for cc in range(CC):
                                    nc.tensor.matmul(
                                        gt_ps[:, 0:n],
                                        gwin[cc][hb % 2][:, hj:hj + P],
                                        xgtb[cc][:, a:b], start=(cc == 0),
                                        stop=(cc == CC - 1))
                                nc.scalar.activation(actr[hc][:, a:b],
                                                     up_ps[:, 0:n], ACTF.Silu)
                                nc.vector.tensor_mul(actr[hc][:, a:b],
                                                     actr[hc][:, a:b],
                                                     gt_ps[:, 0:n])

                    def down_half(h):
                        for sc in range(4 * h, 4 + 5 * h):
                            y_sb = pc_.tile([P, C], BF16, tag="ysb", bufs=2,
                                            name="ysb")
                            for cb in range(2):
                                y_ps = ps_y()
                                for hc in range(HC):
                                    nc.tensor.matmul(
                                        y_ps[:],
                                        actr[hc][:, sc * P:(sc + 1) * P],
                                        rdb[hc][:, cb * 512:(cb + 1) * 512],
                                        start=(hc == 0), stop=(hc == HC - 1))
                                nc.scalar.activation(
                                    y_sb[:, cb * 512:(cb + 1) * 512], y_ps[:],
                                    ACTF.Copy, scale=wslot[:, sc:sc + 1])
                            nc.scalar.dma_start(
                                a2a_in[sc * P:(sc + 1) * P, :], y_sb[:])
                        lo, hi = (0, SH) if h == 0 else (SH, S)
                        nc.gpsimd.collective_compute(
                            "AllToAll", OP.bypass,
                            replica_groups=[list(range(NCORES))],
                            ins=[a2a_in[lo:hi, :]],
                            outs=[a2a_out[lo:hi, :]])

                    with tc.tile_pool(name="pA", bufs=1) as pa:
                        xgt = [pa.tile([P, S], F32, tag=f"xgt{cc}",
                                       name=f"xgt{cc}") for cc in range(CC)]

                        # first up/gate half gets PE going immediately
                        upgate_half(0, 0, SH)
                        nc.sync.dma_start(oh[:], ohx[:])
                        for cc in range(CC):
                            nc.sync.dma_start(rt[cc][:],
                                              rtw[cc * P:(cc + 1) * P, :])
                        for cc in range(CC):
                            nc.sync.dma_start(xgt[cc][:],
                                              xgT[cc * P:(cc + 1) * P, :])

                        lgsb = pa.tile([P, SCH * E], F32, tag="lgsb",
                                       name="lgsb")
                        for sc in range(SCH):
                            lg_ps = ps_y()
                            for cc in range(CC):
                                nc.tensor.matmul(
                                    lg_ps[:, 0:E],
                                    xgt[cc][:, sc * P:(sc + 1) * P],
                                    rt[cc][:],
                                    start=(cc == 0), stop=(cc == CC - 1))
                            nc.vector.tensor_copy(
                                lgsb[:, sc * E:(sc + 1) * E], lg_ps[:, 0:E])

                        def b3(t):
                            return _ap3(t, [[1, SCH], [0, E]])

                        def v3(t):
                            return _ap3(t, [[E, SCH], [1, E]])

                        mx = pa.tile([P, SCH], F32, tag="mx", name="mx")
                        nc.vector.reduce_max(mx[:], v3(lgsb), axis=AX.X)
                        ge1 = pa.tile([P, SCH * E], F32, tag="ge1",
                                      name="ge1")
                        nc.vector.tensor_tensor(v3(ge1), v3(lgsb), b3(mx),
                                                op=OP.is_ge)
                        pen = pa.tile([P, SCH * E], F32, tag="pen",
                                      name="pen")
                        nc.vector.tensor_scalar_mul(pen[:], ge1[:], BIG)
                        l2 = pa.tile([P, SCH * E], F32, tag="l2", name="l2")
                        nc.vector.tensor_tensor(l2[:], lgsb[:], pen[:],
                                                op=OP.subtract)
                        m2 = pa.tile([P, SCH], F32, tag="m2", name="m2")
                        nc.vector.reduce_max(m2[:], v3(l2), axis=AX.X)
                        sel = pa.tile([P, SCH * E], F32, tag="sel",
                                      name="sel")
                        nc.vector.tensor_tensor(v3(sel), v3(lgsb), b3(m2),
                                                op=OP.is_ge)
                        lgs = pa.tile([P, SCH * E], F32, tag="lgs",
                                      name="lgs")
                        nc.vector.tensor_tensor(v3(lgs), v3(lgsb), b3(mx),
                                                op=OP.subtract)
                        ex = pa.tile([P, SCH * E], F32, tag="ex", name="ex")
                        nc.scalar.activation(ex[:], lgs[:], ACTF.Exp)
                        ssum = pa.tile([P, SCH], F32, tag="ssum", name="ssum")
                        nc.vector.reduce_sum(ssum[:], v3(ex), axis=AX.X)
                        exsel = pa.tile([P, SCH * E], F32, tag="exsel",
                                        name="exsel")
                        nc.vector.tensor_mul(exsel[:], ex[:], sel[:])
                        s2 = pa.tile([P, SCH], F32, tag="s2", name="s2")
                        nc.vector.reduce_sum(s2[:], v3(exsel), axis=AX.X)
                        den = pa.tile([P, SCH], F32, tag="den", name="den")
                        nc.vector.tensor_scalar_mul(den[:], ssum[:], 1e-8)
                        nc.vector.tensor_add(den[:], den[:], s2[:])
                        rden = pa.tile([P, SCH], F32, tag="rden", name="rden")
                        nc.vector.reciprocal(rden[:], den[:])
                        wall = pa.tile([P, SCH * E], F32, tag="wall",
                                       name="wall")
                        nc.vector.tensor_tensor(v3(wall), v3(exsel), b3(rden),
                                                op=OP.mult)
                        weo = pa.tile([P, SCH * E], F32, tag="weo",
                                      name="weo")
                        nc.vector.tensor_tensor(
                            v3(weo), v3(wall),
                            _ap3(oh, [[0, SCH], [1, E]]), op=OP.mult)
                        nc.vector.reduce_sum(wslot[:], v3(weo), axis=AX.X)

                        for hc in range(HC):
                            nc.sync.dma_start(rdb[hc][:],
                                              rwd[hc * P:(hc + 1) * P, :])
                        load_rw(0)  # half-1 window reloads (hb 0/1 only;
                        load_rw(1)  # hb 2/3 reload at their hc boundaries)
                        down_half(0)
                        upgate_half(1, SH, S)
                        down_half(1)

                # ---- shared expert (streamed windows, reuse pB-free SBUF) --
                with tc.tile_pool(name="pD", bufs=1) as pd:
                    xsb = [pd.tile([P, TS], BF16, tag=f"xsb{cc}",
                                   name=f"xsb{cc}") for cc in range(CC)]
                    for cc in range(CC):
                        nc.sync.dma_start(xsb[cc][:],
                                          xsTb[cc * P:(cc + 1) * P, :])
                    swin = [[pd.tile([P, 512], BF16, tag=f"sw{cc}_{par}",
                                     name=f"sw{cc}_{par}")
                             for par in range(2)] for cc in range(CC)]
                    twin = [[pd.tile([P, 512], BF16, tag=f"tw{cc}_{par}",
                                     name=f"tw{cc}_{par}")
                             for par in range(2)] for cc in range(CC)]

                    def load_sw(hb):
                        for cc in range(CC):
                            nc.sync.dma_start(
                                swin[cc][hb % 2][:],
                                swu[cc * P:(cc + 1) * P,
                                    hb * 512:(hb + 1) * 512])
                            nc.sync.dma_start(
                                twin[cc][hb % 2][:],
                                swg[cc * P:(cc + 1) * P,
                                    hb * 512:(hb + 1) * 512])

                    load_sw(0)
                    load_sw(1)
                    for hc in range(HC):
                        hb, hj = hc // 4, (hc % 4) * P
                        if hc % 4 == 0 and hb >= 2:
                            load_sw(hb)
                        up_ps, gt_ps = ps_up(), ps_gt()
                        for cc in range(CC):
                            nc.tensor.matmul(up_ps[:],
                                             swin[cc][hb % 2][:, hj:hj + P],
                                             xsb[cc][:], start=(cc == 0),
                                             stop=(cc == CC - 1))
                        for cc in range(CC):
                            nc.tensor.matmul(gt_ps[:],
                                             twin[cc][hb % 2][:, hj:hj + P],
                                             xsb[cc][:], start=(cc == 0),
                                             stop=(cc == CC - 1))
                        nc.scalar.activation(acts[hc][:], up_ps[:], ACTF.Silu)
                        nc.vector.tensor_mul(acts[hc][:], acts[hc][:],
                                             gt_ps[:])

                    # ---- fused combine: shared down + one-hot combine ----
                    with tc.tile_pool(name="pE", bufs=1) as pe:
                        qt = [pe.tile([P, TS], BF16, tag=f"q{sc}",
                                      name=f"q{sc}") for sc in range(SCH)]
                        rv = [pe.tile([P, C], BF16, tag=f"rv{sc}",
                                      name=f"rv{sc}") for sc in range(SCH)]
                        for sc in range(SCH):
                            nc.gpsimd.dma_start(qt[sc][:],
                                                qm[sc * P:(sc + 1) * P, :])
                            nc.gpsimd.dma_start(
                                rv[sc][:], a2a_out[sc * P:(sc + 1) * P, :])
                        wdb = rdb  # reuse the routed-down tiles for swd
                        for hc in range(HC):
                            nc.sync.dma_start(wdb[hc][:],
                                              swd[hc * P:(hc + 1) * P, :])
                        for ts in range(TS // P):
                            for cb in range(2):
                                y_ps = ps_y()
                                for hc in range(HC):
                                    nc.tensor.matmul(
                                        y_ps[:],
                                        acts[hc][:, ts * P:(ts + 1) * P],
                                        wdb[hc][:, cb * 512:(cb + 1) * 512],
                                        start=(hc == 0), stop=False)
                                for sc in range(SCH):
                                    nc.tensor.matmul(
                                        y_ps[:],
                                        qt[sc][:, ts * P:(ts + 1) * P],
                                        rv[sc][:, cb * 512:(cb + 1) * 512],
                                        start=False, stop=(sc == SCH - 1))
                                o_sb = pe.tile([P, 512], F32, tag="osb",
                                               bufs=2, name="osb")
                                nc.vector.tensor_copy(o_sb[:], y_ps[:])
                                nc.scalar.dma_start(
                                    out[ts * P:(ts + 1) * P,
                                        cb * 512:(cb + 1) * 512], o_sb[:])

    nc.compile()
    return nc


_NC_CACHE = None


def kernel(x, shared_Wup, shared_Wgate, shared_Wdown,
           routed_Wup, routed_Wgate, routed_Wdown, router_W):
    global _NC_CACHE
    if _NC_CACHE is None:
        _NC_CACHE = _build_program()
    nc = _NC_CACHE

    xf = np.ascontiguousarray(np.asarray(x, np.float32).reshape(T, C))
    rtw_m = np.ascontiguousarray(np.asarray(router_W, np.float32))

    logits = xf @ rtw_m
    top1 = np.argmax(logits, axis=1)
    l2 = logits.copy()
    l2[np.arange(T), top1] = -np.inf
    top2 = np.argmax(l2, axis=1)

    # balanced ownership: partition tokens into 8 groups of 512 minimizing
    # the max per-(expert, owner) count, so LC can shrink below the worst
    # natural-block load. Greedy + capacity constraint; deterministic.
    cnt = np.zeros((NCORES, E), np.int32)
    cap = np.full(NCORES, TS, np.int32)
    owner = np.empty(T, np.int32)
    order = np.argsort(top1 * E + top2, kind="stable")
    for t in order:
        a, b = top1[t], top2[t]
        best, bo = None, -1
        for o in range(NCORES):
            if cap[o] == 0:
                continue
            key = (max(cnt[o, a] + 1, cnt[o, b] + 1), cnt[o, a] + cnt[o, b],
                   -cap[o])
            if best is None or key < best:
                best, bo = key, o
        owner[t] = bo
        cnt[bo, a] += 1
        cnt[bo, b] += 1
        cap[bo] -= 1
    assert cnt.max() <= LC, f"balance failed: {cnt.max()} > {LC}"
    own_tokens = [np.sort(np.where(owner == o)[0]) for o in range(NCORES)]
    tok_pos = np.empty(T, np.int32)   # local index within owner block
    for o in range(NCORES):
        tok_pos[own_tokens[o]] = np.arange(TS)

    lists = [[[] for _ in range(NCORES)] for _ in range(E)]
    for t in range(T):
        o = owner[t]
        lists[top1[t]][o].append(t)
        lists[top2[t]][o].append(t)

    def slot_of(o, j):
        if j < JH:
            return o * JH + j
        return SH + o * (LC - JH) + (j - JH)

    def b16(a):
        return np.ascontiguousarray(
            np.asarray(a, np.float32).astype(ml_dtypes.bfloat16))

    in_maps = []
    for c in range(NCORES):
        xg = np.zeros((S, C), np.float32)
        for o in range(NCORES):
            for j, t in enumerate(lists[c][o]):
                xg[slot_of(o, j)] = xf[t]
        qmat = np.zeros((S, TS), np.float32)
        for e in range(E):
            for j, t in enumerate(lists[e][c]):
                qmat[slot_of(e, j) if False else (
                    e * JH + j if j < JH else SH + e * (LC - JH) + (j - JH)
                ), tok_pos[t]] = 1.0
        ohv = np.zeros((P, E), np.float32)
        ohv[:, c] = 1.0
        xgt_T = np.ascontiguousarray(xg.T)
        xsT_c = np.ascontiguousarray(xf[own_tokens[c], :].T)
        in_maps.append({
            "xgT": xgt_T,
            "xgTb": b16(xgt_T),
            "rwu": b16(routed_Wup[c]),
            "rwg": b16(routed_Wgate[c]),
            "rwd": b16(routed_Wdown[c]),
            "swu": b16(shared_Wup),
            "swg": b16(shared_Wgate),
            "swd": b16(shared_Wdown),
            "xsTb": b16(xsT_c),
            "rtw": rtw_m,
            "ohx": ohv,
            "qm": b16(qmat),
        })

    res = run_bass_kernel_spmd(nc, in_maps, list(range(NCORES)))
    full = np.empty((T, C), np.float32)
    for c in range(NCORES):
        full[own_tokens[c]] = res.results[c]["out"]
    return full.reshape(2, 2048, C).astype(np.float32)


# revision 10
# speedup vs baseline: 1.0167x; 1.0167x over previous
"""MoE FFN (8 routed experts top-2 + 1 shared) on 8 TRN2 NeuronCores.

Expert-parallel token dispatch (see kernel3 docstring). This version
pipelines the routed expert in two slot halves so the first AllToAll return
overlaps the second half's compute, streams weight tiles through small
reused SBUF windows so the shared-expert loads overlap the routed phase,
and takes the gathered tokens in both fp32 (router precision) and bf16
(matmul operand) form from the host sharding layer.
"""

import ml_dtypes
import numpy as np

import concourse.bacc as bacc
import concourse.bass as bass
import concourse.mybir as mybir
import concourse.tile as tile
from concourse.bass_utils import run_bass_kernel_spmd

P = 128
C = 1024            # d_model
H = 2048            # d_expert
T = 4096            # tokens
E = 8               # routed experts == cores
TS = T // E         # 512 tokens per core
CC = C // P         # 8
HC = H // P         # 16
LC = 144            # per (expert, owner) capacity (host balances ownership)
S = E * LC          # 1280 slots per expert core
SCH = S // P        # 10 slot chunks
JH = 64             # per-owner slots in a2a half 0 (half 1: LC-JH)
SH = E * JH         # 512 rows in a2a half 0
NHB = 4             # 512-wide h blocks
BIG = 1.0e9

F32 = mybir.dt.float32
BF16 = mybir.dt.bfloat16

NCORES = 8
AX = mybir.AxisListType
OP = mybir.AluOpType
ACTF = mybir.ActivationFunctionType


def _ap3(t, dims):
    a = t[:]
    return bass.AP(a.tensor, a.offset, [list(a.ap[0])] + dims)


def _build_program():
    nc = bacc.Bacc("TRN2", target_bir_lowering=False, debug=False,
                   num_devices=NCORES)

    xgT = nc.dram_tensor("xgT", [C, S], F32, kind="ExternalInput")
    xgTb = nc.dram_tensor("xgTb", [C, S], BF16, kind="ExternalInput")
    rwu = nc.dram_tensor("rwu", [C, H], BF16, kind="ExternalInput")
    rwg = nc.dram_tensor("rwg", [C, H], BF16, kind="ExternalInput")
    rwd = nc.dram_tensor("rwd", [H, C], BF16, kind="ExternalInput")
    swu = nc.dram_tensor("swu", [C, H], BF16, kind="ExternalInput")
    swg = nc.dram_tensor("swg", [C, H], BF16, kind="ExternalInput")
    swd = nc.dram_tensor("swd", [H, C], BF16, kind="ExternalInput")
    xsTb = nc.dram_tensor("xsTb", [C, TS], BF16, kind="ExternalInput")
    rtw = nc.dram_tensor("rtw", [C, E], F32, kind="ExternalInput")
    ohx = nc.dram_tensor("ohx", [P, E], F32, kind="ExternalInput")
    qm = nc.dram_tensor("qm", [S, TS], BF16, kind="ExternalInput")

    out = nc.dram_tensor("out", [TS, C], F32, kind="ExternalOutput")

    a2a_in = nc.dram_tensor("a2a_in", [S, C], BF16)
    a2a_out = nc.dram_tensor("a2a_out", [S, C], BF16)

    with tile.TileContext(nc) as tc:
        with (
            tc.tile_pool(name="pers", bufs=1) as pers,
            tc.tile_pool(name="psm", bufs=1, space="PSUM") as psm,
        ):
            rt = [pers.tile([P, E], F32, tag=f"rt{cc}", name=f"rt{cc}")
                  for cc in range(CC)]
            oh = pers.tile([P, E], F32, tag="oh", name="oh")
            wslot = pers.tile([P, SCH], F32, tag="wslot", name="wslot")


            def ps_up():
                return psm.tile([P, 512], F32, tag="up", bufs=2, name="up")

            def ps_gt():
                return psm.tile([P, 512], F32, tag="gt", bufs=2, name="gt")

            def ps_y():
                return psm.tile([P, 512], F32, tag="y", bufs=2, name="y")

            with tc.tile_pool(name="pC", bufs=1) as pc_:
                actr = [pc_.tile([P, S], BF16, tag=f"ar{hc}", name=f"ar{hc}")
                        for hc in range(HC)]
                rdb = [pc_.tile([P, C], BF16, tag=f"rwd{hc}", name=f"rwd{hc}")
                       for hc in range(HC)]
                acts = [pc_.tile([P, TS], BF16, tag=f"as{hc}", name=f"as{hc}")
                        for hc in range(HC)]
                with tc.tile_pool(name="pB", bufs=1) as pb:
                    xgtb = [pb.tile([P, S], BF16, tag=f"xgtb{cc}",
                                    name=f"xgtb{cc}") for cc in range(CC)]
                    # streamed rwu/rwg window: 2 hb-blocks resident
                    rwin = [[pb.tile([P, 512], BF16, tag=f"rw{cc}_{par}",
                                     name=f"rw{cc}_{par}")
                             for par in range(2)] for cc in range(CC)]
                    gwin = [[pb.tile([P, 512], BF16, tag=f"gw{cc}_{par}",
                                     name=f"gw{cc}_{par}")
                             for par in range(2)] for cc in range(CC)]
                    for cc in range(CC):
                        nc.gpsimd.dma_start(xgtb[cc][:, 0:SH],
                                            xgTb[cc * P:(cc + 1) * P, 0:SH])
                    for cc in range(CC):
                        nc.gpsimd.dma_start(xgtb[cc][:, SH:S],
                                            xgTb[cc * P:(cc + 1) * P, SH:S])

                    def load_rw(hb, split=False):
                        for half in ([0, 1] if split else [None]):
                            for cc in range(CC):
                                cl = slice(hb * 512, (hb + 1) * 512) \
                                    if half is None else \
                                    slice(hb * 512 + half * 256,
                                          hb * 512 + (half + 1) * 256)
                                tl = slice(0, 512) if half is None else \
                                    slice(half * 256, (half + 1) * 256)
                                nc.sync.dma_start(
                                    rwin[cc][hb % 2][:, tl],
                                    rwu[cc * P:(cc + 1) * P, cl])
                                nc.sync.dma_start(
                                    gwin[cc][hb % 2][:, tl],
                                    rwg[cc * P:(cc + 1) * P, cl])

                    for cc in range(CC):
                        nc.sync.dma_start(rwin[cc][0][:],
                                          rwu[cc * P:(cc + 1) * P, 0:512])
                        nc.scalar.dma_start(gwin[cc][0][:],
                                            rwg[cc * P:(cc + 1) * P, 0:512])
                    load_rw(1)

                    def upgate_half(h, lo, hi):
                        for hc in range(HC):
                            hb, hj = hc // 4, (hc % 4) * P
                            if hc % 4 == 0 and hc >= 4 and hb + 1 >= 2 \
                                    and hb + 1 <= 3:
                                load_rw(hb + 1)
                            mid = (lo + hi) // 2
                            for sub, (a, b) in enumerate([(lo, mid),
                                                          (mid, hi)]):
                                n = b - a
                                up_ps, gt_ps = ps_up(), ps_gt()
                                for cc in range(CC):
                                    nc.tensor.matmul(
                                        up_ps[:, 0:n],
                                        rwin[cc][hb % 2][:, hj:hj + P],
                                        xgtb[cc][:, a:b], start=(cc == 0),
                                        stop=(cc == CC - 1))
                                for cc in range(CC):
                                    nc.tensor.matmul(
                                        gt_ps[:, 0:n],
                                        gwin[cc][hb % 2][:, hj:hj + P],
                                        xgtb[cc][:, a:b], start=(cc == 0),
                                        stop=(cc == CC - 1))
                                nc.scalar.activation(actr[hc][:, a:b],
                                                     up_ps[:, 0:n], ACTF.Silu)
                                nc.vector.tensor_mul(actr[hc][:, a:b],
                                                     actr[hc][:, a:b],
                                                     gt_ps[:, 0:n])

                    def down_half(h):
                        for sc in range(4 * h, 4 + 5 * h):
                            y_sb = pc_.tile([P, C], BF16, tag="ysb", bufs=2,
                                            name="ysb")
                            for cb in range(2):
                                y_ps = ps_y()
                                for hc in range(HC):
                                    nc.tensor.matmul(
                                        y_ps[:],
                                        actr[hc][:, sc * P:(sc + 1) * P],
                                        rdb[hc][:, cb * 512:(cb + 1) * 512],
                                        start=(hc == 0), stop=(hc == HC - 1))
                                nc.scalar.activation(
                                    y_sb[:, cb * 512:(cb + 1) * 512], y_ps[:],
                                    ACTF.Copy, scale=wslot[:, sc:sc + 1])
                            nc.scalar.dma_start(
                                a2a_in[sc * P:(sc + 1) * P, :], y_sb[:])
                        lo, hi = (0, SH) if h == 0 else (SH, S)
                        nc.gpsimd.collective_compute(
                            "AllToAll", OP.bypass,
                            replica_groups=[list(range(NCORES))],
                            ins=[a2a_in[lo:hi, :]],
                            outs=[a2a_out[lo:hi, :]])

                    with tc.tile_pool(name="pA", bufs=1) as pa:
                        xgt = [pa.tile([P, S], F32, tag=f"xgt{cc}",
                                       name=f"xgt{cc}") for cc in range(CC)]

                        # first up/gate half gets PE going immediately
                        upgate_half(0, 0, SH)
                        nc.sync.dma_start(oh[:], ohx[:])
                        for cc in range(CC):
                            nc.sync.dma_start(rt[cc][:],
                                              rtw[cc * P:(cc + 1) * P, :])
                        for cc in range(CC):
                            nc.sync.dma_start(xgt[cc][:],
                                              xgT[cc * P:(cc + 1) * P, :])

                        lgsb = pa.tile([P, SCH * E], F32, tag="lgsb",
                                       name="lgsb")
                        for sc in range(SCH):
                            lg_ps = ps_y()
                            for cc in range(CC):
                                nc.tensor.matmul(
                                    lg_ps[:, 0:E],
                                    xgt[cc][:, sc * P:(sc + 1) * P],
                                    rt[cc][:],
                                    start=(cc == 0), stop=(cc == CC - 1))
                            nc.vector.tensor_copy(
                                lgsb[:, sc * E:(sc + 1) * E], lg_ps[:, 0:E])

                        def b3(t):
                            return _ap3(t, [[1, SCH], [0, E]])

                        def v3(t):
                            return _ap3(t, [[E, SCH], [1, E]])

                        mx = pa.tile([P, SCH], F32, tag="mx", name="mx")
                        nc.vector.reduce_max(mx[:], v3(lgsb), axis=AX.X)
                        ge1 = pa.tile([P, SCH * E], F32, tag="ge1",
                                      name="ge1")
                        nc.vector.tensor_tensor(v3(ge1), v3(lgsb), b3(mx),
                                                op=OP.is_ge)
                        pen = pa.tile([P, SCH * E], F32, tag="pen",
                                      name="pen")
                        nc.vector.tensor_scalar_mul(pen[:], ge1[:], BIG)
                        l2 = pa.tile([P, SCH * E], F32, tag="l2", name="l2")
                        nc.vector.tensor_tensor(l2[:], lgsb[:], pen[:],
                                                op=OP.subtract)
                        m2 = pa.tile([P, SCH], F32, tag="m2", name="m2")
                        nc.vector.reduce_max(m2[:], v3(l2), axis=AX.X)
                        sel = pa.tile([P, SCH * E], F32, tag="sel",
                                      name="sel")
                        nc.vector.tensor_tensor(v3(sel), v3(lgsb), b3(m2),
                                                op=OP.is_ge)
                        lgs = pa.tile([P, SCH * E], F32, tag="lgs",
                                      name="lgs")
                        nc.vector.tensor_tensor(v3(lgs), v3(lgsb), b3(mx),
                                                op=OP.subtract)
                        ex = pa.tile([P, SCH * E], F32, tag="ex", name="ex")
                        nc.scalar.activation(ex[:], lgs[:], ACTF.Exp)
                        ssum = pa.tile([P, SCH], F32, tag="ssum", name="ssum")
                        nc.vector.reduce_sum(ssum[:], v3(ex), axis=AX.X)
                        exsel = pa.tile([P, SCH * E], F32, tag="exsel",
                                        name="exsel")
                        nc.vector.tensor_mul(exsel[:], ex[:], sel[:])
                        s2 = pa.tile([P, SCH], F32, tag="s2", name="s2")
                        nc.vector.reduce_sum(s2[:], v3(exsel), axis=AX.X)
                        den = pa.tile([P, SCH], F32, tag="den", name="den")
                        nc.vector.tensor_scalar_mul(den[:], ssum[:], 1e-8)
                        nc.vector.tensor_add(den[:], den[:], s2[:])
                        rden = pa.tile([P, SCH], F32, tag="rden", name="rden")
                        nc.vector.reciprocal(rden[:], den[:])
                        wall = pa.tile([P, SCH * E], F32, tag="wall",
                                       name="wall")
                        nc.vector.tensor_tensor(v3(wall), v3(exsel), b3(rden),
                                                op=OP.mult)
                        weo = pa.tile([P, SCH * E], F32, tag="weo",
                                      name="weo")
                        nc.vector.tensor_tensor(
                            v3(weo), v3(wall),
                            _ap3(oh, [[0, SCH], [1, E]]), op=OP.mult)
                        nc.vector.reduce_sum(wslot[:], v3(weo), axis=AX.X)

                        for hc in range(HC):
                            nc.sync.dma_start(rdb[hc][:],
                                              rwd[hc * P:(hc + 1) * P, :])
                        load_rw(0)  # half-1 window reloads (hb 0/1 only;
                        load_rw(1)  # hb 2/3 reload at their hc boundaries)
                        down_half(0)
                        upgate_half(1, SH, S)
                        down_half(1)

                # ---- shared expert (streamed windows, reuse pB-free SBUF) --
                with tc.tile_pool(name="pD", bufs=1) as pd:
                    xsb = [pd.tile([P, TS], BF16, tag=f"xsb{cc}",
                                   name=f"xsb{cc}") for cc in range(CC)]
                    for cc in range(CC):
                        nc.sync.dma_start(xsb[cc][:],
                                          xsTb[cc * P:(cc + 1) * P, :])
                    swin = [[pd.tile([P, 512], BF16, tag=f"sw{cc}_{par}",
                                     name=f"sw{cc}_{par}")
                             for par in range(2)] for cc in range(CC)]
                    twin = [[pd.tile([P, 512], BF16, tag=f"tw{cc}_{par}",
                                     name=f"tw{cc}_{par}")
                             for par in range(2)] for cc in range(CC)]

                    def load_sw(hb):
                        for cc in range(CC):
                            nc.sync.dma_start(
                                swin[cc][hb % 2][:],
                                swu[cc * P:(cc + 1) * P,
                                    hb * 512:(hb + 1) * 512])
                            nc.sync.dma_start(
                                twin[cc][hb % 2][:],
                                swg[cc * P:(cc + 1) * P,
                                    hb * 512:(hb + 1) * 512])

                    load_sw(0)
                    load_sw(1)
                    for hc in range(HC):
                        hb, hj = hc // 4, (hc % 4) * P
                        if hc % 4 == 0 and hb >= 2:
                            load_sw(hb)
                        up_ps, gt_ps = ps_up(), ps_gt()
                        for cc in range(CC):
                            nc.tensor.matmul(up_ps[:],
                                             swin[cc][hb % 2][:, hj:hj + P],
                                             xsb[cc][:], start=(cc == 0),
                                             stop=(cc == CC - 1))
                        for cc in range(CC):
                            nc.tensor.matmul(gt_ps[:],
                                             twin[cc][hb % 2][:, hj:hj + P],
                                             xsb[cc][:], start=(cc == 0),
                                             stop=(cc == CC - 1))
                        nc.scalar.activation(acts[hc][:], up_ps[:], ACTF.Silu)
                        nc.vector.tensor_mul(acts[hc][:], acts[hc][:],
                                             gt_ps[:])

                    # ---- fused combine: shared down + one-hot combine ----
                    with tc.tile_pool(name="pE", bufs=1) as pe:
                        qt = [pe.tile([P, TS], BF16, tag=f"q{sc}",
                                      name=f"q{sc}") for sc in range(SCH)]
                        rv = [pe.tile([P, C], BF16, tag=f"rv{sc}",
                                      name=f"rv{sc}") for sc in range(SCH)]
                        for sc in range(SCH):
                            nc.gpsimd.dma_start(qt[sc][:],
                                                qm[sc * P:(sc + 1) * P, :])
                            nc.gpsimd.dma_start(
                                rv[sc][:], a2a_out[sc * P:(sc + 1) * P, :])
                        wdb = rdb  # reuse the routed-down tiles for swd
                        for hc in range(HC):
                            nc.sync.dma_start(wdb[hc][:],
                                              swd[hc * P:(hc + 1) * P, :])
                        for ts in range(TS // P):
                            for cb in range(2):
                                y_ps = ps_y()
                                for hc in range(HC):
                                    nc.tensor.matmul(
                                        y_ps[:],
                                        acts[hc][:, ts * P:(ts + 1) * P],
                                        wdb[hc][:, cb * 512:(cb + 1) * 512],
                                        start=(hc == 0), stop=False)
                                for sc in range(SCH):
                                    nc.tensor.matmul(
                                        y_ps[:],
                                        qt[sc][:, ts * P:(ts + 1) * P],
                                        rv[sc][:, cb * 512:(cb + 1) * 512],
                                        start=False, stop=(sc == SCH - 1))
                                o_sb = pe.tile([P, 512], F32, tag="osb",
                                               bufs=2, name="osb")
                                nc.vector.tensor_copy(o_sb[:], y_ps[:])
                                nc.scalar.dma_start(
                                    out[ts * P:(ts + 1) * P,
                                        cb * 512:(cb + 1) * 512], o_sb[:])

    nc.compile()
    return nc


_NC_CACHE = None


def kernel(x, shared_Wup, shared_Wgate, shared_Wdown,
           routed_Wup, routed_Wgate, routed_Wdown, router_W):
    global _NC_CACHE
    if _NC_CACHE is None:
        _NC_CACHE = _build_program()
    nc = _NC_CACHE

    xf = np.ascontiguousarray(np.asarray(x, np.float32).reshape(T, C))
    rtw_m = np.ascontiguousarray(np.asarray(router_W, np.float32))

    logits = xf @ rtw_m
    top1 = np.argmax(logits, axis=1)
    l2 = logits.copy()
    l2[np.arange(T), top1] = -np.inf
    top2 = np.argmax(l2, axis=1)

    # balanced ownership: partition tokens into 8 groups of 512 minimizing
    # the max per-(expert, owner) count, so LC can shrink below the worst
    # natural-block load. Greedy + capacity constraint; deterministic.
    cnt = np.zeros((NCORES, E), np.int32)
    cap = np.full(NCORES, TS, np.int32)
    owner = np.empty(T, np.int32)
    order = np.argsort(top1 * E + top2, kind="stable")
    for t in order:
        a, b = top1[t], top2[t]
        best, bo = None, -1
        for o in range(NCORES):
            if cap[o] == 0:
                continue
            key = (max(cnt[o, a] + 1, cnt[o, b] + 1), cnt[o, a] + cnt[o, b],
                   -cap[o])
            if best is None or key < best:
                best, bo = key, o
        owner[t] = bo
        cnt[bo, a] += 1
        cnt[bo, b] += 1
        cap[bo] -= 1
    assert cnt.max() <= LC, f"balance failed: {cnt.max()} > {LC}"
    own_tokens = [np.sort(np.where(owner == o)[0]) for o in range(NCORES)]
    tok_pos = np.empty(T, np.int32)   # local index within owner block
    for o in range(NCORES):
        tok_pos[own_tokens[o]] = np.arange(TS)

    lists = [[[] for _ in range(NCORES)] for _ in range(E)]
    for t in range(T):
        o = owner[t]
        lists[top1[t]][o].append(t)
        lists[top2[t]][o].append(t)

    def slot_of(o, j):
        if j < JH:
            return o * JH + j
        return SH + o * (LC - JH) + (j - JH)

    def b16(a):
        return np.ascontiguousarray(
            np.asarray(a, np.float32).astype(ml_dtypes.bfloat16))

    in_maps = []
    for c in range(NCORES):
        xg = np.zeros((S, C), np.float32)
        for o in range(NCORES):
            for j, t in enumerate(lists[c][o]):
                xg[slot_of(o, j)] = xf[t]
        qmat = np.zeros((S, TS), np.float32)
        for e in range(E):
            for j, t in enumerate(lists[e][c]):
                qmat[slot_of(e, j) if False else (
                    e * JH + j if j < JH else SH + e * (LC - JH) + (j - JH)
                ), tok_pos[t]] = 1.0
        ohv = np.zeros((P, E), np.float32)
        ohv[:, c] = 1.0
        xgt_T = np.ascontiguousarray(xg.T)
        xsT_c = np.ascontiguousarray(xf[own_tokens[c], :].T)
        in_maps.append({
            "xgT": xgt_T,
            "xgTb": b16(xgt_T),
            "rwu": b16(routed_Wup[c]),
            "rwg": b16(routed_Wgate[c]),
            "rwd": b16(routed_Wdown[c]),
            "swu": b16(shared_Wup),
            "swg": b16(shared_Wgate),
            "swd": b16(shared_Wdown),
            "xsTb": b16(xsT_c),
            "rtw": rtw_m,
            "ohx": ohv,
            "qm": b16(qmat),
        })

    res = run_bass_kernel_spmd(nc, in_maps, list(range(NCORES)))
    full = np.empty((T, C), np.float32)
    for c in range(NCORES):
        full[own_tokens[c]] = res.results[c]["out"]
    return full.reshape(2, 2048, C).astype(np.float32)
